# revision 2
# baseline (speedup 1.0000x reference)
"""Trainium2 Bass kernel for nn_MoEFeedForward (dense-MoE with top-2 routing
+ shared expert), distributed over 8 NeuronCores.

Sharding strategy:
  - Experts: expert-parallel, 2 experts per core (of 16).
  - Shared expert: FS (hidden) dim sharded, 256 of 2048 per core.
  - Router: replicated (every core computes probs/top-2 for all tokens).
  - Combine: every core accumulates its partial output for all 2048 tokens
    in token-major layout, then a ReduceScatter(add) over the 8 cores hands
    each core its final 256-token slice. Host concatenates the slices.

Layout convention on device: activations are kept feature-major
([feature, token]) so weight matrices (stored K-major, pre-transposed on
host where needed) are always usable as the stationary matmul operand with
the contraction dim on partitions. Outputs are transposed back to
token-major on the PE before the weighted accumulate, where per-token
scaling is a natural per-partition scalar op.
"""

import sys
import types

import numpy as np

# --- model dims (hardcoded per contract; kernel.py must be self-contained) ---
B, T, D, E, F, FS, TOPK = 2, 1024, 1024, 16, 512, 2048, 2
AUX_COEF = 0.001
NT = B * T            # 2048 tokens
F2 = 2 * F            # 1024
N_CORES = 8
EL = E // N_CORES     # 2 experts per core
FSL = FS // N_CORES   # 256 shared-hidden per core
TSL = NT // N_CORES   # 256 tokens of output per core
P = 128
KD = D // P           # 8 k-tiles over D
KF = F // P           # 4 k-tiles over F
KFS = FSL // P        # 2 k-tiles over the local FS slice
MD = D // P           # 8 m-tiles over D
TT = NT // P          # 16 token tiles
NW = 512              # token chunk (moving free dim)
NCH = NT // NW        # 4 chunks
RW = E + 1            # router cols: 16 experts + shared-gate logit

_NC_CACHE = {}


def _install_ntff_hook():
    """The container's antenv stub lacks axon_hooks; recreate it so
    run_bass_kernel_spmd(trace=True) can reach the NTFF profiler."""
    if "antenv.axon_hooks" in sys.modules:
        return
    try:
        mod = types.ModuleType("antenv.axon_hooks")
        mod._hook = None
        mod.set_axon_ntff_profile_hook = lambda h: setattr(mod, "_hook", h)
        mod.get_axon_ntff_profile_hook = lambda: mod._hook
        sys.modules["antenv.axon_hooks"] = mod
        import antenv

        antenv.axon_hooks = mod
        from trn_agent_boot.trn_boot import _ntff_profile_via_ctypes

        mod.set_axon_ntff_profile_hook(
            _ntff_profile_via_ctypes("/opt/axon/libaxon_pjrt.so")
        )
    except Exception:
        pass


def build_program(use_f32r=False):
    import concourse.mybir as mybir
    import concourse.tile as tile
    from concourse import bacc
    from concourse.masks import make_identity

    f32 = mybir.dt.float32
    A = mybir.AluOpType
    AF = mybir.ActivationFunctionType

    nc = bacc.Bacc("TRN2", target_bir_lowering=False, debug=False,
                   num_devices=N_CORES)

    def cast(ap):
        return ap.bitcast(mybir.dt.float32r) if use_f32r else ap

    # ---- I/O ----
    xT_d = nc.dram_tensor("xT", [D, NT], f32, kind="ExternalInput").ap()
    w1t_d = nc.dram_tensor("w1t", [EL, D, F2], f32, kind="ExternalInput").ap()
    w2t_d = nc.dram_tensor("w2t", [EL, F, D], f32, kind="ExternalInput").ap()
    shg_d = nc.dram_tensor("shg", [D, FSL], f32, kind="ExternalInput").ap()
    shu_d = nc.dram_tensor("shu", [D, FSL], f32, kind="ExternalInput").ap()
    shd_d = nc.dram_tensor("shd", [FSL, D], f32, kind="ExternalInput").ap()
    rw_d = nc.dram_tensor("rw17", [D, RW], f32, kind="ExternalInput").ap()
    eid_d = nc.dram_tensor("eids", [P, EL], f32, kind="ExternalInput").ap()

    y_d = nc.dram_tensor("y", [TSL, D], f32, kind="ExternalOutput").ap()
    aux_d = nc.dram_tensor("aux", [1, 1], f32, kind="ExternalOutput").ap()

    rs_in = nc.dram_tensor("rs_in", [NT, D], f32).ap()
    rs_out = nc.dram_tensor("rs_out", [TSL, D], f32).ap()

    with tile.TileContext(nc) as tc:
        with (
            tc.tile_pool(name="big", bufs=1) as big,
            tc.tile_pool(name="const", bufs=1) as cst,
            tc.tile_pool(name="psum", bufs=4, space="PSUM") as pp,
            tc.tile_pool(name="psumT", bufs=2, space="PSUM") as ppt,
            tc.tile_pool(name="psumacc", bufs=1, space="PSUM") as ppa,
        ):
            # ---- persistent SBUF ----
            xt = big.tile([P, KD * NT], f32)        # xT, 8 k-tiles [128, 2048]
            moe = big.tile([P, TT * D], f32)        # token-major accumulator
            ident = cst.tile([P, P], f32)
            make_identity(nc, ident[:])
            ones = cst.tile([P, 1], f32)
            nc.vector.memset(ones[:], 1.0)
            wloc = cst.tile([P, TT * EL], f32)      # per-core expert weights
            gate = cst.tile([P, TT], f32)           # shared-expert sigmoid gate
            eid = cst.tile([P, EL], f32)
            nc.sync.dma_start(out=eid[:], in_=eid_d[:])

            for kd in range(KD):
                nc.sync.dma_start(out=xt[:, kd * NT:(kd + 1) * NT],
                                  in_=xT_d[kd * P:(kd + 1) * P, :])

            # ================= router (token-major) =================
            with tc.tile_pool(name="rt", bufs=2) as wk:
                rw = cst.tile([P, KD * RW], f32)
                for kd in range(KD):
                    nc.sync.dma_start(out=rw[:, kd * RW:(kd + 1) * RW],
                                      in_=rw_d[kd * P:(kd + 1) * P, :])

                load_ps = ppa.tile([E, 1], f32, tag="acc")
                for tt in range(TT):
                    ps = pp.tile([P, RW], f32, tag="mm")
                    for kd in range(KD):
                        nc.tensor.matmul(
                            ps[:],
                            lhsT=cast(xt[:, kd * NT + tt * P: kd * NT + tt * P + P]),
                            rhs=cast(rw[:, kd * RW:(kd + 1) * RW]),
                            start=(kd == 0), stop=(kd == KD - 1))
                    # sigmoid for the shared-expert gate (col 16)
                    nc.scalar.activation(gate[:, tt:tt + 1], ps[:, E:E + 1],
                                         AF.Sigmoid)
                    # softmax over the 16 expert logits
                    probs = wk.tile([P, E], f32, tag="probs")
                    rmax = wk.tile([P, 4], f32, tag="stat")
                    nc.vector.tensor_reduce(out=rmax[:, 0:1], in_=ps[:, 0:E],
                                            axis=mybir.AxisListType.X, op=A.max)
                    nc.vector.tensor_scalar_mul(rmax[:, 1:2], rmax[:, 0:1], -1.0)
                    nc.scalar.activation(probs[:], ps[:, 0:E], AF.Exp,
                                         bias=rmax[:, 1:2])
                    nc.vector.tensor_reduce(out=rmax[:, 2:3], in_=probs[:],
                                            axis=mybir.AxisListType.X, op=A.add)
                    nc.vector.reciprocal(rmax[:, 3:4], rmax[:, 2:3])
                    nc.vector.tensor_scalar_mul(probs[:], probs[:], rmax[:, 3:4])
                    # aux-loss load accumulation: load_ps += probs^T @ ones
                    nc.tensor.matmul(load_ps[:], lhsT=probs[:], rhs=ones[:],
                                     start=(tt == 0), stop=(tt == TT - 1))
                    # top-2 values + indices
                    mx = wk.tile([P, 8], f32, tag="mx")
                    mxi = wk.tile([P, 8], mybir.dt.uint32, tag="mxi")
                    nc.vector.max(out=mx[:], in_=probs[:])
                    nc.vector.max_index(out=mxi[:], in_max=mx[:],
                                        in_values=probs[:])
                    st = wk.tile([P, 8], f32, tag="st")
                    # st: 0=vsum 1=rcp 2=w0 3=w1 4=idx0 5=idx1
                    nc.vector.tensor_add(st[:, 0:1], mx[:, 0:1], mx[:, 1:2])
                    nc.vector.tensor_scalar_max(st[:, 0:1], st[:, 0:1], 1e-9)
                    nc.vector.reciprocal(st[:, 1:2], st[:, 0:1])
                    nc.vector.tensor_mul(st[:, 2:3], mx[:, 0:1], st[:, 1:2])
                    nc.vector.tensor_mul(st[:, 3:4], mx[:, 1:2], st[:, 1:2])
                    nc.vector.tensor_copy(st[:, 4:6], mxi[:, 0:2])  # u32->f32
                    # wloc[t, l] = w0*(idx0==eid_l) + w1*(idx1==eid_l)
                    cmp = wk.tile([P, 4], f32, tag="cmp")
                    for l in range(EL):
                        nc.vector.tensor_tensor(cmp[:, 0:1], st[:, 4:5],
                                                eid[:, l:l + 1], op=A.is_equal)
                        nc.vector.tensor_tensor(cmp[:, 1:2], st[:, 5:6],
                                                eid[:, l:l + 1], op=A.is_equal)
                        nc.vector.tensor_mul(cmp[:, 2:3], cmp[:, 0:1],
                                             st[:, 2:3])
                        nc.vector.tensor_mul(cmp[:, 3:4], cmp[:, 1:2],
                                             st[:, 3:4])
                        nc.vector.tensor_add(
                            wloc[:, tt * EL + l: tt * EL + l + 1],
                            cmp[:, 2:3], cmp[:, 3:4])

                # aux loss tail: aux = coef * sum_e (load/NT - 1/E)^2
                lsb = wk.tile([E, 4], f32, tag="lsb")
                nc.vector.tensor_scalar(lsb[:, 0:1], load_ps[:], 1.0 / NT,
                                        -1.0 / E, op0=A.mult, op1=A.add)
                nc.vector.tensor_mul(lsb[:, 1:2], lsb[:, 0:1], lsb[:, 0:1])
                aux_ps = ppa.tile([1, 1], f32, tag="aux")
                nc.tensor.matmul(aux_ps[:], lhsT=lsb[:, 1:2], rhs=ones[0:E, :],
                                 start=True, stop=True)
                aux_sb = wk.tile([1, 1], f32, tag="auxsb")
                nc.scalar.activation(aux_sb[:], aux_ps[:], AF.Copy,
                                     scale=AUX_COEF)
                nc.sync.dma_start(out=aux_d[:], in_=aux_sb[:])

            # ================= shared expert (FS-sliced) =================
            with (
                tc.tile_pool(name="shw", bufs=1) as swp,
                tc.tile_pool(name="shwork", bufs=2) as wk,
            ):
                shg = swp.tile([P, KD * FSL], f32, tag="shg")
                shu = swp.tile([P, KD * FSL], f32, tag="shu")
                shd = swp.tile([P, KFS * D], f32, tag="shd")
                for kd in range(KD):
                    nc.sync.dma_start(out=shg[:, kd * FSL:(kd + 1) * FSL],
                                      in_=shg_d[kd * P:(kd + 1) * P, :])
                    nc.sync.dma_start(out=shu[:, kd * FSL:(kd + 1) * FSL],
                                      in_=shu_d[kd * P:(kd + 1) * P, :])
                for kf in range(KFS):
                    nc.sync.dma_start(out=shd[:, kf * D:(kf + 1) * D],
                                      in_=shd_d[kf * P:(kf + 1) * P, :])

                for n in range(NCH):
                    hsh = wk.tile([P, KFS * NW], f32, tag="hsh")  # [fs, t]
                    for m in range(KFS):
                        psg = pp.tile([P, NW], f32, tag="mm")
                        psu = pp.tile([P, NW], f32, tag="mm")
                        for kd in range(KD):
                            nc.tensor.matmul(
                                psg[:],
                                lhsT=cast(shg[:, kd * FSL + m * P: kd * FSL + (m + 1) * P]),
                                rhs=cast(xt[:, kd * NT + n * NW: kd * NT + (n + 1) * NW]),
                                start=(kd == 0), stop=(kd == KD - 1))
                            nc.tensor.matmul(
                                psu[:],
                                lhsT=cast(shu[:, kd * FSL + m * P: kd * FSL + (m + 1) * P]),
                                rhs=cast(xt[:, kd * NT + n * NW: kd * NT + (n + 1) * NW]),
                                start=(kd == 0), stop=(kd == KD - 1))
                        sil = wk.tile([P, NW], f32, tag="sil")
                        nc.scalar.activation(sil[:], psg[:], AF.Silu)
                        nc.vector.tensor_tensor(
                            hsh[:, m * NW:(m + 1) * NW],
                            sil[:], psu[:], op=A.mult)
                    for md in range(MD):
                        psd = pp.tile([P, NW], f32, tag="mm")
                        for kf in range(KFS):
                            nc.tensor.matmul(
                                psd[:],
                                lhsT=cast(shd[:, kf * D + md * P: kf * D + (md + 1) * P]),
                                rhs=cast(hsh[:, kf * NW:(kf + 1) * NW]),
                                start=(kf == 0), stop=(kf == KFS - 1))
                        sdf = wk.tile([P, NW], f32, tag="sdf")
                        nc.scalar.activation(sdf[:], psd[:], AF.Copy)
                        for j in range(NW // P):
                            tt = n * (NW // P) + j
                            pst = ppt.tile([P, P], f32, tag="tr")
                            nc.tensor.transpose(pst[:],
                                                sdf[:, j * P:(j + 1) * P],
                                                ident[:])
                            # moe[token tile tt, d cols md] = gate * shared^T
                            nc.vector.tensor_scalar_mul(
                                moe[:, tt * D + md * P: tt * D + (md + 1) * P],
                                pst[:], gate[:, tt:tt + 1])

            # ================= experts (dense over local 2) =================
            with (
                tc.tile_pool(name="ew", bufs=1) as ewp,
                tc.tile_pool(name="ework", bufs=2) as wk,
            ):
                for e in range(EL):
                    w1 = ewp.tile([P, KD * F2], f32, tag="w1")
                    for kd in range(KD):
                        nc.sync.dma_start(out=w1[:, kd * F2:(kd + 1) * F2],
                                          in_=w1t_d[e, kd * P:(kd + 1) * P, :])
                    w2 = ewp.tile([P, KF * D], f32, tag="w2")
                    for kf in range(KF):
                        nc.sync.dma_start(out=w2[:, kf * D:(kf + 1) * D],
                                          in_=w2t_d[e, kf * P:(kf + 1) * P, :])
                    for n in range(NCH):
                        eh = wk.tile([P, KF * NW], f32, tag="eh")  # [f, t]
                        for mf in range(KF):
                            psg = pp.tile([P, NW], f32, tag="mm")
                            psu = pp.tile([P, NW], f32, tag="mm")
                            for kd in range(KD):
                                nc.tensor.matmul(
                                    psg[:],
                                    lhsT=cast(w1[:, kd * F2 + mf * P: kd * F2 + (mf + 1) * P]),
                                    rhs=cast(xt[:, kd * NT + n * NW: kd * NT + (n + 1) * NW]),
                                    start=(kd == 0), stop=(kd == KD - 1))
                                nc.tensor.matmul(
                                    psu[:],
                                    lhsT=cast(w1[:, kd * F2 + (KF + mf) * P: kd * F2 + (KF + mf + 1) * P]),
                                    rhs=cast(xt[:, kd * NT + n * NW: kd * NT + (n + 1) * NW]),
                                    start=(kd == 0), stop=(kd == KD - 1))
                            sil = wk.tile([P, NW], f32, tag="sil")
                            nc.scalar.activation(sil[:], psg[:], AF.Silu)
                            nc.vector.tensor_tensor(
                                eh[:, mf * NW:(mf + 1) * NW],
                                sil[:], psu[:], op=A.mult)
                        for md in range(MD):
                            psd = pp.tile([P, NW], f32, tag="mm")
                            for kf in range(KF):
                                nc.tensor.matmul(
                                    psd[:],
                                    lhsT=cast(w2[:, kf * D + md * P: kf * D + (md + 1) * P]),
                                    rhs=cast(eh[:, kf * NW:(kf + 1) * NW]),
                                    start=(kf == 0), stop=(kf == KF - 1))
                            yo = wk.tile([P, NW], f32, tag="yo")
                            nc.scalar.activation(yo[:], psd[:], AF.Copy)
                            for j in range(NW // P):
                                tt = n * (NW // P) + j
                                pst = ppt.tile([P, P], f32, tag="tr")
                                nc.tensor.transpose(
                                    pst[:], yo[:, j * P:(j + 1) * P], ident[:])
                                tmp = wk.tile([P, P], f32, tag="tmp")
                                nc.vector.tensor_scalar_mul(
                                    tmp[:], pst[:],
                                    wloc[:, tt * EL + e: tt * EL + e + 1])
                                nc.vector.tensor_add(
                                    moe[:, tt * D + md * P: tt * D + (md + 1) * P],
                                    moe[:, tt * D + md * P: tt * D + (md + 1) * P],
                                    tmp[:])

            # ================= combine: ReduceScatter =================
            with tc.tile_pool(name="fin", bufs=2) as fnp:
                for tt in range(TT):
                    nc.sync.dma_start(out=rs_in[tt * P:(tt + 1) * P, :],
                                      in_=moe[:, tt * D:(tt + 1) * D])
                nc.gpsimd.collective_compute(
                    "ReduceScatter", A.add,
                    replica_groups=[list(range(N_CORES))],
                    ins=[rs_in[:]], outs=[rs_out[:]])
                for i in range(TSL // P):
                    fin = fnp.tile([P, D], f32, tag="fin")
                    nc.sync.dma_start(out=fin[:],
                                      in_=rs_out[i * P:(i + 1) * P, :])
                    nc.sync.dma_start(out=y_d[i * P:(i + 1) * P, :],
                                      in_=fin[:])

    nc.compile()
    return nc


def make_in_maps(x, gate_up_proj, down_proj, router_w, sh_gate_w, sh_up_w,
                 sh_down_w, shared_gate_w):
    x2 = np.ascontiguousarray(x.reshape(NT, D))
    xT = np.ascontiguousarray(x2.T)
    rw17 = np.ascontiguousarray(
        np.concatenate([router_w, shared_gate_w], axis=1))
    in_maps = []
    for c in range(N_CORES):
        es = slice(c * EL, (c + 1) * EL)
        fs = slice(c * FSL, (c + 1) * FSL)
        in_maps.append({
            "xT": xT,
            "w1t": np.ascontiguousarray(
                gate_up_proj[es].transpose(0, 2, 1)),
            "w2t": np.ascontiguousarray(down_proj[es].transpose(0, 2, 1)),
            "shg": np.ascontiguousarray(sh_gate_w[:, fs]),
            "shu": np.ascontiguousarray(sh_up_w[:, fs]),
            "shd": np.ascontiguousarray(sh_down_w[fs, :]),
            "rw17": rw17,
            "eids": np.tile(
                np.arange(c * EL, (c + 1) * EL, dtype=np.float32), (P, 1)),
        })
    return in_maps


def kernel(x, gate_up_proj, down_proj, router_w, sh_gate_w, sh_up_w,
           sh_down_w, shared_gate_w, _trace=False, _use_f32r=True,
           _tmpdir=None):
    _install_ntff_hook()
    from concourse.bass_utils import run_bass_kernel_spmd

    key = ("nc", _use_f32r)
    if key not in _NC_CACHE:
        _NC_CACHE[key] = build_program(use_f32r=_use_f32r)
    nc = _NC_CACHE[key]

    in_maps = make_in_maps(
        np.asarray(x, dtype=np.float32),
        np.asarray(gate_up_proj, dtype=np.float32),
        np.asarray(down_proj, dtype=np.float32),
        np.asarray(router_w, dtype=np.float32),
        np.asarray(sh_gate_w, dtype=np.float32),
        np.asarray(sh_up_w, dtype=np.float32),
        np.asarray(sh_down_w, dtype=np.float32),
        np.asarray(shared_gate_w, dtype=np.float32),
    )
    res = run_bass_kernel_spmd(nc, in_maps, list(range(N_CORES)),
                               trace=_trace, tmpdir=_tmpdir)
    y = np.concatenate([res.results[c]["y"] for c in range(N_CORES)], axis=0)
    out = y.reshape(B, T, D)
    aux = np.float32(res.results[0]["aux"][0, 0])
    if _trace:
        kernel._last = res
    return out, aux


# revision 9
# speedup vs baseline: 2.3023x; 2.3023x over previous
"""Trainium2 Bass kernel for nn_MoEFeedForward (dense-MoE with top-2 routing
+ shared expert), distributed over 8 NeuronCores.

Sharding strategy:
  - Experts: expert-parallel, 2 experts per core (of 16).
  - Shared expert: FS (hidden) dim sharded, 256 of 2048 per core.
  - Router: replicated (every core computes probs/top-2 for all tokens).
  - Combine: every core accumulates its partial output for all 2048 tokens
    in token-major layout, then a ReduceScatter(add) over the 8 cores hands
    each core its final 256-token slice. Host concatenates the slices.

Layout convention on device: activations are kept feature-major
([feature, token]) so weight matrices (stored K-major, pre-transposed on
host where needed) are always usable as the stationary matmul operand with
the contraction dim on partitions. Outputs are transposed back to
token-major on the PE before the weighted accumulate, where per-token
scaling is a natural per-partition scalar op.
"""

import sys
import types

import numpy as np

# --- model dims (hardcoded per contract; kernel.py must be self-contained) ---
B, T, D, E, F, FS, TOPK = 2, 1024, 1024, 16, 512, 2048, 2
AUX_COEF = 0.001
NT = B * T            # 2048 tokens
F2 = 2 * F            # 1024
N_CORES = 8
EL = E // N_CORES     # 2 experts per core
FSL = FS // N_CORES   # 256 shared-hidden per core
TSL = NT // N_CORES   # 256 tokens of output per core
P = 128
KD = D // P           # 8 k-tiles over D
KF = F // P           # 4 k-tiles over F
KFS = FSL // P        # 2 k-tiles over the local FS slice
MD = D // P           # 8 m-tiles over D
TT = NT // P          # 16 token tiles
NW = 512              # token chunk (moving free dim)
NCH = NT // NW        # 4 chunks
RW = E + 2            # router cols: 16 experts + gate logit + pad (f32r needs even N)

_NC_CACHE = {}


def _install_ntff_hook():
    """The container's antenv stub lacks axon_hooks; recreate it so
    run_bass_kernel_spmd(trace=True) can reach the NTFF profiler."""
    if "antenv.axon_hooks" in sys.modules:
        return
    try:
        mod = types.ModuleType("antenv.axon_hooks")
        mod._hook = None
        mod.set_axon_ntff_profile_hook = lambda h: setattr(mod, "_hook", h)
        mod.get_axon_ntff_profile_hook = lambda: mod._hook
        sys.modules["antenv.axon_hooks"] = mod
        import antenv

        antenv.axon_hooks = mod
        from trn_agent_boot.trn_boot import _ntff_profile_via_ctypes

        mod.set_axon_ntff_profile_hook(
            _ntff_profile_via_ctypes("/opt/axon/libaxon_pjrt.so")
        )
    except Exception:
        pass


def build_program(use_f32r=False, debug_taps=False):
    import concourse.mybir as mybir
    import concourse.tile as tile
    from concourse import bacc
    from concourse.masks import make_identity

    f32 = mybir.dt.float32
    A = mybir.AluOpType
    AF = mybir.ActivationFunctionType

    nc = bacc.Bacc("TRN2", target_bir_lowering=False, debug=False,
                   num_devices=N_CORES)
    dt_m = mybir.dt.float32r if use_f32r else f32

    # ---- I/O ----
    xT_d = nc.dram_tensor("xT", [D, NT], dt_m, kind="ExternalInput").ap()
    w1t_d = nc.dram_tensor("w1t", [EL, D, F2], dt_m, kind="ExternalInput").ap()
    w2t_d = nc.dram_tensor("w2t", [EL, F, D], dt_m, kind="ExternalInput").ap()
    shg_d = nc.dram_tensor("shg", [D, FSL], dt_m, kind="ExternalInput").ap()
    shu_d = nc.dram_tensor("shu", [D, FSL], dt_m, kind="ExternalInput").ap()
    shd_d = nc.dram_tensor("shd", [FSL, D], dt_m, kind="ExternalInput").ap()
    rw_d = nc.dram_tensor("rw17", [D, RW], f32, kind="ExternalInput").ap()
    eid_d = nc.dram_tensor("eids", [P, EL], f32, kind="ExternalInput").ap()

    y_d = nc.dram_tensor("y", [TSL, D], f32, kind="ExternalOutput").ap()
    aux_d = nc.dram_tensor("aux", [1, 1], f32, kind="ExternalOutput").ap()

    rs_in = nc.dram_tensor("rs_in", [NT, D], f32).ap()
    if debug_taps:
        dbg_wloc = nc.dram_tensor("dbg_wloc", [P, TT * EL], f32,
                                  kind="ExternalOutput").ap()
        dbg_gate = nc.dram_tensor("dbg_gate", [P, TT], f32,
                                  kind="ExternalOutput").ap()
        dbg_rsin = nc.dram_tensor("dbg_rsin", [NT, D], f32,
                                  kind="ExternalOutput").ap()
    rs_out = nc.dram_tensor("rs_out", [TSL, D], f32).ap()

    with tile.TileContext(nc) as tc:
        with (
            tc.tile_pool(name="big", bufs=1) as big,
            tc.tile_pool(name="const", bufs=1) as cst,
            tc.tile_pool(name="psum", bufs=4, space="PSUM") as pp,
            tc.tile_pool(name="psumT", bufs=2, space="PSUM") as ppt,
            tc.tile_pool(name="psumacc", bufs=1, space="PSUM") as ppa,
        ):
            # ---- persistent SBUF ----
            xt = big.tile([P, KD * NT], dt_m)        # xT, 8 k-tiles [128, 2048]
            moe = big.tile([P, TT * D], f32)        # token-major accumulator
            ident = cst.tile([P, P], f32)
            make_identity(nc, ident[:])
            ones = cst.tile([P, 1], f32)
            nc.vector.memset(ones[:], 1.0)
            wloc = cst.tile([P, TT * EL], f32)      # per-core expert weights
            gate = cst.tile([P, TT], f32)           # shared-expert sigmoid gate
            eid = cst.tile([P, EL], f32)
            nc.sync.dma_start(out=eid[:], in_=eid_d[:])

            for kd in range(KD):
                nc.sync.dma_start(out=xt[:, kd * NT:(kd + 1) * NT],
                                  in_=xT_d[kd * P:(kd + 1) * P, :])

            # ================= router (token-major) =================
            with tc.tile_pool(name="rt", bufs=2) as wk:
                rw = cst.tile([P, KD * RW], f32)
                for kd in range(KD):
                    nc.sync.dma_start(out=rw[:, kd * RW:(kd + 1) * RW],
                                      in_=rw_d[kd * P:(kd + 1) * P, :])

                load_ps = ppa.tile([E, 1], f32, tag="acc")
                for tt in range(TT):
                    ps = pp.tile([P, RW], f32, tag="mm")
                    for kd in range(KD):
                        nc.tensor.matmul(
                            ps[:],
                            lhsT=xt[:, kd * NT + tt * P: kd * NT + tt * P + P].bitcast(f32),
                            rhs=rw[:, kd * RW:(kd + 1) * RW],
                            start=(kd == 0), stop=(kd == KD - 1))
                    # sigmoid for the shared-expert gate (col 16)
                    nc.scalar.activation(gate[:, tt:tt + 1], ps[:, E:E + 1],
                                         AF.Sigmoid)
                    # softmax over the 16 expert logits
                    probs = wk.tile([P, E], f32, tag="probs")
                    rmax = wk.tile([P, 4], f32, tag="stat")
                    nc.vector.tensor_reduce(out=rmax[:, 0:1], in_=ps[:, 0:E],
                                            axis=mybir.AxisListType.X, op=A.max)
                    nc.vector.tensor_scalar_mul(rmax[:, 1:2], rmax[:, 0:1], -1.0)
                    nc.scalar.activation(probs[:], ps[:, 0:E], AF.Exp,
                                         bias=rmax[:, 1:2])
                    nc.vector.tensor_reduce(out=rmax[:, 2:3], in_=probs[:],
                                            axis=mybir.AxisListType.X, op=A.add)
                    nc.vector.reciprocal(rmax[:, 3:4], rmax[:, 2:3])
                    nc.vector.tensor_scalar_mul(probs[:], probs[:], rmax[:, 3:4])
                    # aux-loss load accumulation: load_ps += probs^T @ ones
                    nc.tensor.matmul(load_ps[:], lhsT=probs[:], rhs=ones[:],
                                     start=(tt == 0), stop=(tt == TT - 1))
                    # top-2 values + indices
                    mx = wk.tile([P, 8], f32, tag="mx")
                    mxi = wk.tile([P, 8], mybir.dt.uint32, tag="mxi")
                    nc.vector.max(out=mx[:], in_=probs[:])
                    nc.vector.max_index(out=mxi[:], in_max=mx[:],
                                        in_values=probs[:])
                    st = wk.tile([P, 8], f32, tag="st")
                    # st: 0=vsum 1=rcp 2=w0 3=w1 4=idx0 5=idx1
                    nc.vector.tensor_add(st[:, 0:1], mx[:, 0:1], mx[:, 1:2])
                    nc.vector.tensor_scalar_max(st[:, 0:1], st[:, 0:1], 1e-9)
                    nc.vector.reciprocal(st[:, 1:2], st[:, 0:1])
                    nc.vector.tensor_mul(st[:, 2:3], mx[:, 0:1], st[:, 1:2])
                    nc.vector.tensor_mul(st[:, 3:4], mx[:, 1:2], st[:, 1:2])
                    nc.vector.tensor_copy(st[:, 4:6], mxi[:, 0:2])  # u32->f32
                    # wloc[t, l] = w0*(idx0==eid_l) + w1*(idx1==eid_l)
                    cmp = wk.tile([P, 4], f32, tag="cmp")
                    for l in range(EL):
                        nc.vector.tensor_tensor(cmp[:, 0:1], st[:, 4:5],
                                                eid[:, l:l + 1], op=A.is_equal)
                        nc.vector.tensor_tensor(cmp[:, 1:2], st[:, 5:6],
                                                eid[:, l:l + 1], op=A.is_equal)
                        nc.vector.tensor_mul(cmp[:, 2:3], cmp[:, 0:1],
                                             st[:, 2:3])
                        nc.vector.tensor_mul(cmp[:, 3:4], cmp[:, 1:2],
                                             st[:, 3:4])
                        nc.vector.tensor_add(
                            wloc[:, tt * EL + l: tt * EL + l + 1],
                            cmp[:, 2:3], cmp[:, 3:4])

                # aux loss tail: aux = coef * sum_e (load/NT - 1/E)^2
                lsb = wk.tile([E, 4], f32, tag="lsb")
                nc.vector.tensor_scalar(lsb[:, 0:1], load_ps[:], 1.0 / NT,
                                        -1.0 / E, op0=A.mult, op1=A.add)
                nc.vector.tensor_mul(lsb[:, 1:2], lsb[:, 0:1], lsb[:, 0:1])
                aux_ps = ppa.tile([1, 1], f32, tag="aux")
                nc.tensor.matmul(aux_ps[:], lhsT=lsb[:, 1:2], rhs=ones[0:E, :],
                                 start=True, stop=True)
                aux_sb = wk.tile([1, 1], f32, tag="auxsb")
                nc.scalar.activation(aux_sb[:], aux_ps[:], AF.Copy,
                                     scale=AUX_COEF)
                nc.sync.dma_start(out=aux_d[:], in_=aux_sb[:])

            # ================= shared expert (FS-sliced) =================
            with (
                tc.tile_pool(name="shw", bufs=1) as swp,
                tc.tile_pool(name="shwork", bufs=2) as wk,
            ):
                shg = swp.tile([P, KD * FSL], dt_m, tag="shg")
                shu = swp.tile([P, KD * FSL], dt_m, tag="shu")
                shd = swp.tile([P, KFS * D], dt_m, tag="shd")
                for kd in range(KD):
                    nc.sync.dma_start(out=shg[:, kd * FSL:(kd + 1) * FSL],
                                      in_=shg_d[kd * P:(kd + 1) * P, :])
                    nc.sync.dma_start(out=shu[:, kd * FSL:(kd + 1) * FSL],
                                      in_=shu_d[kd * P:(kd + 1) * P, :])
                for kf in range(KFS):
                    nc.sync.dma_start(out=shd[:, kf * D:(kf + 1) * D],
                                      in_=shd_d[kf * P:(kf + 1) * P, :])

                for n in range(NCH):
                    hsh = wk.tile([P, KFS * NW], dt_m, tag="hsh")  # [fs, t]
                    for m in range(KFS):
                        psg = pp.tile([P, NW], f32, tag="mm")
                        psu = pp.tile([P, NW], f32, tag="mm")
                        for kd in range(KD):
                            nc.tensor.matmul(
                                psg[:],
                                lhsT=shg[:, kd * FSL + m * P: kd * FSL + (m + 1) * P],
                                rhs=xt[:, kd * NT + n * NW: kd * NT + (n + 1) * NW],
                                start=(kd == 0), stop=(kd == KD - 1))
                            nc.tensor.matmul(
                                psu[:],
                                lhsT=shu[:, kd * FSL + m * P: kd * FSL + (m + 1) * P],
                                rhs=xt[:, kd * NT + n * NW: kd * NT + (n + 1) * NW],
                                start=(kd == 0), stop=(kd == KD - 1))
                        sil = wk.tile([P, NW], f32, tag="sil")
                        nc.scalar.activation(sil[:], psg[:], AF.Silu)
                        nc.vector.tensor_tensor(
                            hsh[:, m * NW:(m + 1) * NW],
                            sil[:], psu[:], op=A.mult)
                    for md in range(MD):
                        psd = pp.tile([P, NW], f32, tag="mm")
                        for kf in range(KFS):
                            nc.tensor.matmul(
                                psd[:],
                                lhsT=shd[:, kf * D + md * P: kf * D + (md + 1) * P],
                                rhs=hsh[:, kf * NW:(kf + 1) * NW],
                                start=(kf == 0), stop=(kf == KFS - 1))
                        sdf = wk.tile([P, NW], f32, tag="sdf")
                        nc.scalar.activation(sdf[:], psd[:], AF.Copy)
                        for j in range(NW // P):
                            tt = n * (NW // P) + j
                            pst = ppt.tile([P, P], f32, tag="tr")
                            nc.tensor.transpose(pst[:],
                                                sdf[:, j * P:(j + 1) * P],
                                                ident[:])
                            # moe[token tile tt, d cols md] = gate * shared^T
                            nc.vector.tensor_scalar_mul(
                                moe[:, tt * D + md * P: tt * D + (md + 1) * P],
                                pst[:], gate[:, tt:tt + 1])

            # ================= experts (dense over local 2) =================
            with (
                tc.tile_pool(name="ew", bufs=1) as ewp,
                tc.tile_pool(name="ework", bufs=2) as wk,
            ):
                for e in range(EL):
                    w1 = ewp.tile([P, KD * F2], dt_m, tag="w1")
                    for kd in range(KD):
                        nc.sync.dma_start(out=w1[:, kd * F2:(kd + 1) * F2],
                                          in_=w1t_d[e, kd * P:(kd + 1) * P, :])
                    w2 = ewp.tile([P, KF * D], dt_m, tag="w2")
                    for kf in range(KF):
                        nc.sync.dma_start(out=w2[:, kf * D:(kf + 1) * D],
                                          in_=w2t_d[e, kf * P:(kf + 1) * P, :])
                    for n in range(NCH):
                        eh = wk.tile([P, KF * NW], dt_m, tag="eh")  # [f, t]
                        for mf in range(KF):
                            psg = pp.tile([P, NW], f32, tag="mm")
                            psu = pp.tile([P, NW], f32, tag="mm")
                            for kd in range(KD):
                                nc.tensor.matmul(
                                    psg[:],
                                    lhsT=w1[:, kd * F2 + mf * P: kd * F2 + (mf + 1) * P],
                                    rhs=xt[:, kd * NT + n * NW: kd * NT + (n + 1) * NW],
                                    start=(kd == 0), stop=(kd == KD - 1))
                                nc.tensor.matmul(
                                    psu[:],
                                    lhsT=(w1[:, kd * F2 + (KF + mf) * P: kd * F2 + (KF + mf + 1) * P]),
                                    rhs=xt[:, kd * NT + n * NW: kd * NT + (n + 1) * NW],
                                    start=(kd == 0), stop=(kd == KD - 1))
                            sil = wk.tile([P, NW], f32, tag="sil")
                            nc.scalar.activation(sil[:], psg[:], AF.Silu)
                            nc.vector.tensor_tensor(
                                eh[:, mf * NW:(mf + 1) * NW],
                                sil[:], psu[:], op=A.mult)
                        for md in range(MD):
                            psd = pp.tile([P, NW], f32, tag="mm")
                            for kf in range(KF):
                                nc.tensor.matmul(
                                    psd[:],
                                    lhsT=w2[:, kf * D + md * P: kf * D + (md + 1) * P],
                                    rhs=eh[:, kf * NW:(kf + 1) * NW],
                                    start=(kf == 0), stop=(kf == KF - 1))
                            yo = wk.tile([P, NW], f32, tag="yo")
                            nc.scalar.activation(yo[:], psd[:], AF.Copy)
                            for j in range(NW // P):
                                tt = n * (NW // P) + j
                                pst = ppt.tile([P, P], f32, tag="tr")
                                nc.tensor.transpose(
                                    pst[:], yo[:, j * P:(j + 1) * P], ident[:])
                                tmp = wk.tile([P, P], f32, tag="tmp")
                                nc.vector.tensor_scalar_mul(
                                    tmp[:], pst[:],
                                    wloc[:, tt * EL + e: tt * EL + e + 1])
                                nc.vector.tensor_add(
                                    moe[:, tt * D + md * P: tt * D + (md + 1) * P],
                                    moe[:, tt * D + md * P: tt * D + (md + 1) * P],
                                    tmp[:])

            # ================= combine: ReduceScatter =================
            with tc.tile_pool(name="fin", bufs=2) as fnp:
                if debug_taps:
                    nc.sync.dma_start(out=dbg_wloc[:], in_=wloc[:])
                    nc.sync.dma_start(out=dbg_gate[:], in_=gate[:])
                    for tt in range(TT):
                        nc.sync.dma_start(
                            out=dbg_rsin[tt * P:(tt + 1) * P, :],
                            in_=moe[:, tt * D:(tt + 1) * D])
                for tt in range(TT):
                    nc.sync.dma_start(out=rs_in[tt * P:(tt + 1) * P, :],
                                      in_=moe[:, tt * D:(tt + 1) * D])
                nc.gpsimd.collective_compute(
                    "ReduceScatter", A.add,
                    replica_groups=[list(range(N_CORES))],
                    ins=[rs_in[:]], outs=[rs_out[:]])
                for i in range(TSL // P):
                    fin = fnp.tile([P, D], f32, tag="fin")
                    nc.sync.dma_start(out=fin[:],
                                      in_=rs_out[i * P:(i + 1) * P, :])
                    nc.sync.dma_start(out=y_d[i * P:(i + 1) * P, :],
                                      in_=fin[:])

    nc.compile()
    return nc


def make_in_maps(x, gate_up_proj, down_proj, router_w, sh_gate_w, sh_up_w,
                 sh_down_w, shared_gate_w):
    x2 = np.ascontiguousarray(x.reshape(NT, D))
    xT = np.ascontiguousarray(x2.T)
    rw17 = np.ascontiguousarray(np.concatenate(
        [router_w, shared_gate_w,
         np.zeros((D, 1), dtype=np.float32)], axis=1))
    in_maps = []
    for c in range(N_CORES):
        es = slice(c * EL, (c + 1) * EL)
        fs = slice(c * FSL, (c + 1) * FSL)
        in_maps.append({
            "xT": xT,
            "w1t": np.ascontiguousarray(
                gate_up_proj[es].transpose(0, 2, 1)),
            "w2t": np.ascontiguousarray(down_proj[es].transpose(0, 2, 1)),
            "shg": np.ascontiguousarray(sh_gate_w[:, fs]),
            "shu": np.ascontiguousarray(sh_up_w[:, fs]),
            "shd": np.ascontiguousarray(sh_down_w[fs, :]),
            "rw17": rw17,
            "eids": np.tile(
                np.arange(c * EL, (c + 1) * EL, dtype=np.float32), (P, 1)),
        })
    return in_maps


def kernel(x, gate_up_proj, down_proj, router_w, sh_gate_w, sh_up_w,
           sh_down_w, shared_gate_w, _trace=False, _use_f32r=True,
           _tmpdir=None):
    _install_ntff_hook()
    from concourse.bass_utils import run_bass_kernel_spmd

    key = ("nc", _use_f32r)
    if key not in _NC_CACHE:
        _NC_CACHE[key] = build_program(use_f32r=_use_f32r)
    nc = _NC_CACHE[key]

    in_maps = make_in_maps(
        np.asarray(x, dtype=np.float32),
        np.asarray(gate_up_proj, dtype=np.float32),
        np.asarray(down_proj, dtype=np.float32),
        np.asarray(router_w, dtype=np.float32),
        np.asarray(sh_gate_w, dtype=np.float32),
        np.asarray(sh_up_w, dtype=np.float32),
        np.asarray(sh_down_w, dtype=np.float32),
        np.asarray(shared_gate_w, dtype=np.float32),
    )
    res = run_bass_kernel_spmd(nc, in_maps, list(range(N_CORES)),
                               trace=_trace, tmpdir=_tmpdir)
    y = np.concatenate([res.results[c]["y"] for c in range(N_CORES)], axis=0)
    out = y.reshape(B, T, D)
    aux = np.float32(res.results[0]["aux"][0, 0])
    if _trace:
        kernel._last = res
    return out, aux
